# revision 24
# baseline (speedup 1.0000x reference)
"""LocallyConnected1d Trainium2 kernel, v29 (SBUF-resident, single DMA row,
PSUM quadrant packing).  ~51 us vs the 55-57 us v15 baseline.

Problem: out[b, oc, w] = sum_{ic,k} xp[b, ic, w+k] * W[w, oc, ic, k] + bias[oc, w]
  x: (32, 64, 2048) f32, weights: (2048, 64, 64, 3) f32, bias: (64, 2048) f32
  out: (32, 64, 2048) f32.  xp = x padded by 1 on both sides of the last axis.

Sharding: output_width (2048) split into 8 chunks of 256, one per core.

Design (what measured faster, and why):
  1. Full SBUF residency: every input slice has its own tile (no pool
     reuse), so all input DMAs are enqueued up front and stream
     back-to-back; compute chases the completion sems.  (v15's bufs=4
     pools gated DMA issue on compute progress and starved the stream.)
  2. Single DMA queue row: the 16 SDMA engines round-robin between queue
     rows at packet granularity, which caps each at ~17 GB/s when 3 rows
     are active (~23 GB/s with one).  All transfers ride the sync
     (HWDGE) row in consumption order, so arrival is perfectly paced;
     only the head slice fans its three ~0.7us DIRECT2D triggers across
     sync/scalar/gpsimd so the first matmul isn't trigger-serialized.
  3. PSUM quadrant packing: a superchunk = 32 positions = 4 chunks of 8;
     chunk c's matmuls write PSUM partitions [32c:32c+32) of one
     [128, 8, 64] bank via tile_position=(0, 32c) (the 4th col group
     must be explicit).  PSUM->SBUF evacuation is one full-width copy
     per 32 positions instead of four quarter-width ones.
  4. Output DRAM layout [128, 8, 8, 64] = (32c+b, s, wi, oc), position
     w = 32s + 8c + wi, decoded during the host unshard (which also adds
     bias).  Keeps out-DMA descriptors at 1KB contiguous runs.
  5. Slices tapered (8,24,64,64,48,24,16,8): small head starts the MM
     stream after ~0.26 MB; big middle maximizes transfer efficiency;
     small tail shrinks the last-input -> last-output critical path.

Rejected (measured slower or failing): 3 balanced DMA rows (engine
round-robin cost), SBUF->SBUF x-dedup (SDMA engines, not HBM, are the
bottleneck - same engine bytes), batched 128-col LDWEIGHTS with
ldweights=False matmuls (implicit LDWs still emitted + slow tiled mode),
fp8 weights (rel err 2.2e-2 > 2e-2 gate), per-slice contiguous DRAM.

Compute per position (2 matmuls, from v15):
  xs[j, t, b] = xp[b, j%64, ws+t + j//64]      [128, 258, 32]
  mm1: psum[p] += xs[:, p].T     @ w01[:, p]   (K=128: k0+k1)
  mm2: psum[p] += xs[0:65, p+2].T @ w2[:, p]   (K=65; w2 row 64 zero)
"""

import ml_dtypes
import numpy as np

import concourse.bacc as bacc
import concourse.mybir as mybir
import concourse.tile as tile
from concourse.bass_utils import run_bass_kernel_spmd

B, IC, OC, KS, W = 32, 64, 64, 3, 2048
NCORES = 8
OWC = W // NCORES  # 256 positions per core
CH = 8             # positions per chunk; 4 chunks -> one [128, 8, 64] PSUM bank
SC = 4 * CH        # superchunk: 32 positions
NSC = OWC // SC    # 8 superchunks per core
BF16 = mybir.dt.bfloat16
F32 = mybir.dt.float32
NPBF16 = ml_dtypes.bfloat16

_compiled_nc = None

SLICES = (8, 24, 64, 64, 48, 24, 16, 8)
assert sum(SLICES) == OWC
OUT_GROUPS_SC = (2, 2, 2, 1, 1)  # in superchunks; sums to NSC
assert sum(OUT_GROUPS_SC) == NSC


def _build_nc():
    nc = bacc.Bacc("TRN2")

    xs_d = nc.dram_tensor("xs", [2 * IC, OWC + 2, B], BF16, kind="ExternalInput")
    w01_d = nc.dram_tensor("w01", [2 * IC, OWC, OC], BF16, kind="ExternalInput")
    w2_d = nc.dram_tensor("w2", [IC + 1, OWC, OC], BF16, kind="ExternalInput")
    # out_d[32c+b, s, wi, oc] = out[b, oc, 32s + 8c + wi]
    out_d = nc.dram_tensor("out", [4 * B, NSC, CH, OC], BF16, kind="ExternalOutput")

    starts = []
    p = 0
    for s in SLICES:
        starts.append(p)
        p += s

    with tile.TileContext(nc) as tc:
        with (
            tc.tile_pool(name="w", bufs=1) as wpool,
            tc.tile_pool(name="x", bufs=1) as xpool,
            tc.tile_pool(name="o", bufs=4) as opool,
            tc.tile_pool(name="ps", bufs=8, space="PSUM") as pspool,
        ):
            loaded = []

            # Issue every input DMA up front; tiles are never reused so the
            # three queue rows stream continuously at HBM line rate.  The SDMA
            # engines round-robin packets across rows with pending work, so a
            # row gets ~1/3 of the bandwidth: rotate each tensor's slices
            # across the rows so every row carries ~11 KB/position and
            # positions complete in lockstep (w01 alone is 16.4 KB/pos and
            # starves the matmul stream if pinned to one row).
            # Single queue row for all DMAs: SDMA engines round-robin at
            # *packet* granularity across every row with pending work, which
            # costs per-engine throughput when several rows are active
            # (measured ~17 GB/s/engine with 3 rows vs ~23+ with one), and a
            # single FIFO row delivers slices strictly in position order so
            # per-position arrival is perfectly paced.  The system is SDMA-
            # engine-bound, not HBM-bound: SBUF->SBUF copies cost the same
            # engine time as HBM loads (measured, v21), so the k1-shifted x
            # copy stays as a second HBM load baked into xs_d host-side.
            for si, plen in enumerate(SLICES):
                p0 = starts[si]
                sl = slice(p0, p0 + plen)
                w01 = wpool.tile([2 * IC, plen, OC], BF16, tag=f"w01_{si}",
                                 name=f"w01_{si}")
                w2 = wpool.tile([IC + 1, plen, OC], BF16, tag=f"w2_{si}",
                                name=f"w2_{si}")
                xs = xpool.tile([2 * IC, plen + 2, B], BF16, tag=f"xs_{si}",
                                name=f"xs_{si}")
                if si == 0:
                    # The three ~0.6-1.0us DIRECT2D triggers serialize on one
                    # engine, which would delay slice 0's third transfer (and
                    # the first matmul) by ~2.3us.  Fan the head slice's
                    # triggers across the three DMA-capable engines; nothing
                    # else is streaming yet, so the multi-row round-robin
                    # penalty doesn't apply.
                    nc.sync.dma_start(out=w01[:], in_=w01_d[:, sl, :])
                    nc.scalar.dma_start(out=w2[:], in_=w2_d[:, sl, :])
                    nc.gpsimd.dma_start(
                        out=xs[:], in_=xs_d[:, p0 : p0 + plen + 2, :]
                    )
                else:
                    nc.sync.dma_start(out=w01[:], in_=w01_d[:, sl, :])
                    nc.sync.dma_start(out=w2[:], in_=w2_d[:, sl, :])
                    nc.sync.dma_start(
                        out=xs[:], in_=xs_d[:, p0 : p0 + plen + 2, :]
                    )
                loaded.append((p0, plen, w01, w2, xs))

            group_of_sc = {}
            s0 = 0
            for g in OUT_GROUPS_SC:
                group_of_sc[s0] = g
                s0 += g

            ob = None
            ob_sc0 = 0
            ps = None
            for si in range(len(SLICES)):
                p0, plen, w01, w2, xs = loaded[si]
                for c0 in range(0, plen, CH):
                    g0 = p0 + c0            # global position of this chunk
                    sc = g0 // SC           # superchunk index
                    q = (g0 % SC) // CH     # quadrant -> PSUM partitions 32q..
                    if ob is None:
                        ob_glen = group_of_sc[sc]
                        ob = opool.tile([4 * B, ob_glen, CH, OC], BF16, tag="ob",
                                        name=f"ob_{sc}")
                        ob_sc0 = sc
                    if ps is None:
                        ps = pspool.tile([4 * B, CH, OC], F32, tag="ps",
                                         name=f"ps_{sc}")
                    for wi in range(CH):
                        wl = c0 + wi
                        nc.tensor.matmul(
                            ps[32 * q : 32 * q + 32, wi, :],
                            xs[:, wl, :],
                            w01[:, wl, :],
                            start=True,
                            stop=False,
                            tile_position=(0, 32 * q),
                        )
                        nc.tensor.matmul(
                            ps[32 * q : 32 * q + 32, wi, :],
                            xs[0 : IC + 1, wl + 2, :],
                            w2[:, wl, :],
                            start=False,
                            stop=True,
                            tile_position=(0, 32 * q),
                        )
                    if q == 3:
                        nc.vector.tensor_copy(
                            out=ob[:, sc - ob_sc0, :, :], in_=ps[:]
                        )
                        ps = None
                        if sc + 1 - ob_sc0 == ob_glen:
                            nc.sync.dma_start(
                                out=out_d[:, ob_sc0 : ob_sc0 + ob_glen, :, :],
                                in_=ob[:],
                            )
                            ob = None

    nc.compile()
    return nc


def _get_nc():
    global _compiled_nc
    if _compiled_nc is None:
        _compiled_nc = _build_nc()
    return _compiled_nc


def shard_inputs(x, weights, bias):
    x = np.asarray(x, dtype=np.float32)
    weights = np.asarray(weights, dtype=np.float32)

    xp = np.pad(x, ((0, 0), (0, 0), (1, 1)))
    xpT = np.ascontiguousarray(xp.transpose(1, 2, 0)).astype(NPBF16)  # (IC, W+2, B)
    wT = weights.transpose(3, 2, 0, 1).astype(NPBF16)  # (KS, IC, W, OC)

    in_maps = []
    for c in range(NCORES):
        ws = c * OWC
        xs = np.concatenate(
            [xpT[:, ws : ws + OWC + 2, :], xpT[:, ws + 1 : ws + OWC + 3, :]], axis=0
        ) if ws + OWC + 3 <= W + 2 else np.concatenate(
            [
                xpT[:, ws : ws + OWC + 2, :],
                np.pad(xpT[:, ws + 1 :, :], ((0, 0), (0, ws + OWC + 3 - (W + 2)), (0, 0))),
            ],
            axis=0,
        )
        w01 = np.concatenate(
            [wT[0, :, ws : ws + OWC, :], wT[1, :, ws : ws + OWC, :]], axis=0
        )
        in_maps.append(
            {
                "xs": np.ascontiguousarray(xs),
                "w01": np.ascontiguousarray(w01),
                "w2": np.ascontiguousarray(np.concatenate(
                    [wT[2, :, ws : ws + OWC, :], np.zeros((1, OWC, OC), NPBF16)],
                    axis=0,
                )),
            }
        )
    return in_maps


def run_sharded(x, weights, bias, trace=False):
    nc = _get_nc()
    in_maps = shard_inputs(x, weights, bias)
    res = run_bass_kernel_spmd(nc, in_maps, list(range(NCORES)), trace=trace)
    bias = np.asarray(bias, dtype=np.float32)
    out = np.empty((B, OC, W), np.float32)
    for c in range(NCORES):
        r = res.results[c]["out"].astype(np.float32)  # (128, NSC, CH, OC)
        r = r.reshape(4, B, NSC, CH, OC)              # (c, b, s, wi, oc)
        r = r.transpose(1, 2, 0, 3, 4).reshape(B, OWC, OC)  # (b, w, oc)
        out[:, :, c * OWC : (c + 1) * OWC] = r.transpose(0, 2, 1)
    out += bias[None, :, :]
    return out, res


def kernel(x, weights, bias):
    out, _ = run_sharded(x, weights, bias)
    return out
